# revision 27
# baseline (speedup 1.0000x reference)
"""Trainium2 Bass kernel for the latent-query attention module.

Module math (fp32 inputs):
  Q = latent @ Wq.T; K = data @ Wk.T; V = data @ Wv.T
  S = (Q K^T)/sqrt(D); P = softmax_keys(S); out = (P V) @ Wo.T + bo

Sharding: 8 cores = 4 batches x 2 head-groups (4 heads each). Each core
computes Q/K/V for its heads, full attention over all 4096 keys and all
512 queries, and a PARTIAL output projection attn_g @ Wo[:, g].T.
Host gather sums the two partials per batch, divides by 16 (Wk/Wv ship
x16, below) and adds the bias.

Cost-model-aware design (graded time = concourse TimelineSim); this is
the baseline schedule plus two pure-PE reductions:
  - K/V projections run as fp8e4m3 DoubleRow (0.5 cycles/row AND 2x128
    contraction per instruction) using host-side hi/lo error-feedback
    splits of data and of 16x-scaled Wk/Wv (16x keeps the lo residues
    out of e4m3's subnormal range):  K ~ dhi@whi + dlo@whi + dhi@wlo.
    3 DR instructions replace 4 bf16 ones at better-than-bf16 accuracy.
  - Attention stays law-split BY HEAD (softmax purity): heads 0,3 use
    ACT true Exp emitting P directly as e4m3 (scale/16, bias -1); their
    PV runs as key-block-PAIRED fp8 DoubleRow against an e4m3 copy of
    V (+ones) that the otherwise-idle GPSIMD converts SBUF->SBUF from
    the bf16 V. 4x fewer PE cycles for those heads' PV. Heads 1,2 keep
    the DVE Schraudolph bf16 exp and the bf16 65-wide PV, so their
    softmax stays at bf16 precision (rel-err ~1.1e-2 total, tol 2e-2).
  - The ones column makes PV col 64 the softmax denominator (the 1.0 is
    not Wv-scaled, so att carries a clean 16x removed on host).
"""

import sys

sys.path.insert(0, "/opt/trn_rl_repo")

import numpy as np

B, DS, DC = 4, 4096, 256
LS, LC = 512, 512
H, D = 8, 64
INNER, OUT_DIM = 512, 512
NCORES = 8
HPC = 4                 # heads per core
IH = HPC * D            # inner half = 256
KB = DS // 128          # 32 key blocks
NPAIR = KB // 2         # 16 key-block pairs
NCH = DS // 512         # 8 data chunks
SCALE = D ** -0.5
WSCALE = 16.0           # Wk/Wv host-side scale (hi/lo fp8 subnormal fix)

# Schraudolph exp for bf16 bit patterns: bf16bits(exp(s)) ~ EA*s + EB
# (SCALE and the 1/16 K-scale folded into EA).
EA = 128.0 * 1.4426950408889634 * SCALE / WSCALE
EB = 16256.0 - 5.59 + 0.5
# ACT heads: P = exp(s*SCALE/16 - C_ACT) in e4m3 (max ~90 << 240)
C_ACT = 1.0

DR_HEADS = (0, 3)       # ACT true-exp heads -> e4m3 DoubleRow PV
_CACHE = {}


def _emit(ctx, tc, nc):
    from concourse import mybir

    f32 = mybir.dt.float32
    bf16 = mybir.dt.bfloat16
    e4 = mybir.dt.float8e4
    i16 = mybir.dt.int16
    Exp = mybir.ActivationFunctionType.Exp
    MUL = mybir.AluOpType.mult
    ADD = mybir.AluOpType.add
    DR = mybir.MatmulPerfMode.DoubleRow

    # ---- DRAM I/O (partition-major; see shard()) ----
    latentT = nc.dram_tensor("latentT", [128, 4, LS], bf16, kind="ExternalInput").ap()
    wqT = nc.dram_tensor("wqT", [128, 4, IH], bf16, kind="ExternalInput").ap()
    dhi = nc.dram_tensor("dhi", [128, 2, DS], e4, kind="ExternalInput").ap()
    dlo = nc.dram_tensor("dlo", [128, 2, DS], e4, kind="ExternalInput").ap()
    wkhi = nc.dram_tensor("wkhi", [128, 2, IH], e4, kind="ExternalInput").ap()
    wklo = nc.dram_tensor("wklo", [128, 2, IH], e4, kind="ExternalInput").ap()
    wvhi = nc.dram_tensor("wvhi", [128, 2, IH], e4, kind="ExternalInput").ap()
    wvlo = nc.dram_tensor("wvlo", [128, 2, IH], e4, kind="ExternalInput").ap()
    woT = nc.dram_tensor("woT", [128, 2, OUT_DIM], bf16, kind="ExternalInput").ap()
    ident = nc.dram_tensor("ident", [128, 128], bf16, kind="ExternalInput").ap()
    outp = nc.dram_tensor("outp", [128, 4, OUT_DIM], bf16,
                          kind="ExternalOutput").ap()

    # ---- resident SBUF ----
    res = ctx.enter_context(tc.tile_pool(name="res", bufs=1))
    kt = res.tile([128, 2, DS], bf16, name="kt")        # K^T head-pairs
    v_r = res.tile([128, KB, HPC, 65], bf16, name="v")  # V + ones col
    v8 = res.tile([128, KB, 2, 65], e4, name="v8")      # e4m3 V, DR heads
    qt = res.tile([128, HPC, LS], bf16, name="qt")      # Q^T zero-padded
    att = res.tile([128, 4, 2, 128], bf16, name="att")  # normalized [q, i]
    attnT = res.tile([128, 2, 4, 128], bf16, name="attnT")
    cbias = res.tile([128, 1], f32, name="cbias")
    wts = ctx.enter_context(tc.tile_pool(name="wts", bufs=1))
    lat_s = wts.tile([128, 4, LS], bf16, name="lat_s")
    wq_s = wts.tile([128, 4, IH], bf16, name="wq_s")
    wkh_s = wts.tile([128, 2, IH], e4, name="wkh_s")
    wkl_s = wts.tile([128, 2, IH], e4, name="wkl_s")
    wvh_s = wts.tile([128, 2, IH], e4, name="wvh_s")
    wvl_s = wts.tile([128, 2, IH], e4, name="wvl_s")
    wo_s = wts.tile([128, 2, OUT_DIM], bf16, name="wo_s")
    id_s = wts.tile([128, 128], bf16, name="id_s")

    # input DMAs, spread over issuing engines so chunk 0 arrives ASAP:
    # SP: the 8 data chunk pairs (chunk 0 first); Pool (SWDGE): weights;
    # ACT: latent early, wo/id behind the qt copies.
    nc.gpsimd.dma_start(wkh_s[:], wkhi)
    nc.gpsimd.dma_start(wkl_s[:], wklo)
    nc.gpsimd.dma_start(wq_s[:], wqT)
    nc.gpsimd.dma_start(wvh_s[:], wvhi)
    nc.gpsimd.dma_start(wvl_s[:], wvlo)
    nc.scalar.dma_start(lat_s[:], latentT)
    nc.scalar.dma_start(wo_s[:], woT)
    nc.scalar.dma_start(id_s[:], ident)

    # ---- PE warmup: ~3us of dummy matmuls during the DMA lead-in so
    # the cost model's p-state ramp finishes before real work arrives. ----
    wu = res.tile([128, 72], bf16, name="wu")
    nc.vector.memset(wu[:], 0.0)
    nc.vector.memset(cbias[:], -C_ACT)
    with tc.tile_pool(name="wps", bufs=1, space="PSUM") as wps:
        wp = wps.tile([8, 64], f32, name="wp")
        for _ in range(60):
            nc.tensor.matmul(wp[:], wu[:, 0:8], wu[:, 8:72],
                             start=True, stop=True)
    nc.gpsimd.memset(qt[:], 0.0)
    nc.gpsimd.memset(v_r[:, :, :, 64:65], 1.0)

    # exp engine schedule per head-slot (GPSIMD cannot read PSUM on hw):
    # ACT true Exp -> e4m3 pair tiles for heads 0/3; DVE Schraudolph
    # -> bf16 for heads 1/2.
    EXP_ENG = [0, 1, 1, 0]
    ptp = ctx.enter_context(tc.tile_pool(name="ptp", bufs=3))
    pt8p = ctx.enter_context(tc.tile_pool(name="pt8p", bufs=2))
    pt8cur = {}

    def exp_kb(kb, h, s_ap):
        """Returns the P tile for (kb, h): bf16 per-kb tile for sch heads,
        e4m3 pair tile (allocated on even kb) for DR heads."""
        if EXP_ENG[h] == 0:
            if kb % 2 == 0:
                pt8cur[h] = pt8p.tile([128, 2, 512], e4, tag=f"pt8{h}",
                                      name=f"pt8{h}")
            pt = pt8cur[h]
            nc.scalar.activation(pt[:, kb % 2, :], s_ap, Exp,
                                 scale=SCALE / WSCALE, bias=cbias[:])
            return pt
        pt = ptp.tile([128, 512], bf16, tag=f"pt{h}", name=f"pt{h}")
        nc.vector.tensor_scalar(pt[:].bitcast(i16), s_ap, EA, EB, MUL, ADD)
        return pt

    early_pts = {}

    # ---- phases 0+1: Q^T after chunk 0, K^T/V streamed over 8 chunks ----
    # sEp=2: key-blocks 0..3's S+exp units (16) run in phase-1 ACT/DVE
    # slack, cutting 4 rounds off the DVE-exp-paced phase 2.
    with tc.tile_pool(name="dstage", bufs=3) as dstage, \
         tc.tile_pool(name="kvps", bufs=2, space="PSUM") as kvps, \
         tc.tile_pool(name="sEp", bufs=1, space="PSUM") as sEp, \
         tc.tile_pool(name="vps", bufs=3, space="PSUM") as vps:

        def load_chunk(ch):
            dh = dstage.tile([128, 2, 512], e4, tag="dh", name="dh")
            dl = dstage.tile([128, 2, 512], e4, tag="dl", name="dl")
            nc.sync.dma_start(dh[:], dhi[:, :, ch * 512:(ch + 1) * 512])
            nc.sync.dma_start(dl[:], dlo[:, :, ch * 512:(ch + 1) * 512])
            return dh, dl

        def kv_proj(ch, dh, dl, v_first=False):
            def k_part():
                # K^T = 16*(Wk data^T): 3-term hi/lo fp8 DoubleRow
                kp = kvps.tile([128, 2, 512], f32, tag="kp", name="kp")
                for m in range(2):
                    mc = slice(m * 128, (m + 1) * 128)
                    nc.tensor.matmul(kp[:, m, :], wkh_s[:, :, mc], dh[:],
                                     start=True, stop=False, perf_mode=DR)
                    nc.tensor.matmul(kp[:, m, :], wkh_s[:, :, mc], dl[:],
                                     start=False, stop=False, perf_mode=DR)
                    nc.tensor.matmul(kp[:, m, :], wkl_s[:, :, mc], dh[:],
                                     start=False, stop=True, perf_mode=DR)
                nc.scalar.copy(kt[:, :, ch * 512:(ch + 1) * 512], kp[:])
            if not v_first:
                k_part()
            for k2 in range(2):
                # V = 16*(data Wv^T): per key-128-block 3-term DR
                vp = vps.tile([128, 2, IH], f32, tag="vp", name="vp")
                for i in range(2):
                    ic = slice((2 * k2 + i) * 128, (2 * k2 + i + 1) * 128)
                    nc.tensor.matmul(vp[:, i, :], dh[:, :, ic], wvh_s[:],
                                     start=True, stop=False, perf_mode=DR)
                    nc.tensor.matmul(vp[:, i, :], dl[:, :, ic], wvh_s[:],
                                     start=False, stop=False, perf_mode=DR)
                    nc.tensor.matmul(vp[:, i, :], dh[:, :, ic], wvl_s[:],
                                     start=False, stop=True, perf_mode=DR)
                use_act = (v_first and k2 == 0) or (not v_first and ch % 3 == 1 and k2 == 1)
                eng = nc.scalar if use_act else nc.vector
                dst = v_r[:, ch * 4 + 2 * k2:ch * 4 + 2 * k2 + 2, :, 0:64]
                src = vp[:].rearrange("p b (h e) -> p b h e", e=64)
                if eng is nc.scalar:
                    eng.copy(dst, src)
                else:
                    eng.tensor_copy(dst, src)
            # idle GPSIMD converts this chunk's DR-head V to e4m3 (incl
            # the ones column written by the memset above)
            nc.gpsimd.tensor_copy(v8[:, 4 * ch:4 * ch + 4, :, :],
                                  v_r[:, 4 * ch:4 * ch + 4, 0::3, :])
            if v_first:
                k_part()

        drs = [load_chunk(0), load_chunk(1), load_chunk(2)]
        kv_proj(0, *drs[0])
        kv_proj(1, *drs[1])
        drs.append(load_chunk(3))
        kv_proj(2, *drs[2])
        # Q^T projection into the zero-padded per-head copies
        qp = kvps.tile([128, 2, 512], f32, tag="kp", name="kp")
        for m in range(2):
            for c in range(4):
                nc.tensor.matmul(qp[:, m, :],
                                 wq_s[:, c, m * 128:(m + 1) * 128],
                                 lat_s[:, c, :], start=(c == 0), stop=(c == 3))
            # rows 0:64 = head 2m, rows 64:128 = head 2m+1
            nc.scalar.copy(qt[0:64, 2 * m, :], qp[0:64, m, :])
            nc.scalar.copy(qt[64:128, 2 * m + 1, :], qp[64:128, m, :])
        # prefill schedule: key-block 0's S+exp units, one per chunk
        PREFILL = {3: [(0, 0)], 4: [(0, 1)], 5: [(0, 2)], 6: [(0, 3)],
                   7: []}

        def prefill(kb, h):
            sE = sEp.tile([128, 512], f32, tag="se", name="se")
            nc.tensor.matmul(sE[:], kt[:, h // 2, kb * 128:(kb + 1) * 128],
                             qt[:, h, :], start=True, stop=True)
            early_pts[(kb, h)] = exp_kb(kb, h, sE[:])

        for ch in range(3, NCH):
            if ch + 1 < NCH:
                drs.append(load_chunk(ch + 1))
            kv_proj(ch, *drs[ch], v_first=(ch == NCH - 1))
            for kb, h in PREFILL[ch]:
                prefill(kb, h)

    # ---- phase 2: attention (S -> exp -> PV), streamed over key blocks ----
    if True:
        pvps_ctx = tc.tile_pool(name="pvps", bufs=1, space="PSUM")
        pvps = pvps_ctx.__enter__()
        sps_ctx = tc.tile_pool(name="sps", bufs=1, space="PSUM")
        sps = sps_ctx.__enter__()
        pv = [pvps.tile([128, 4, 65], f32, name=f"pv{h}") for h in range(HPC)]
        prev = None

        def emit_s(kb, h):
            s_ = sps.tile([128, 512], f32, tag=f"s{h}", name=f"s{h}")
            nc.tensor.matmul(s_[:], kt[:, h // 2, kb * 128:(kb + 1) * 128],
                             qt[:, h, :], start=True, stop=True)
            return exp_kb(kb, h, s_[:])

        def emit_pv(kb, h, pt, qbs=range(4)):
            if EXP_ENG[h] == 0:
                if kb % 2 == 0:
                    return  # pair incomplete; fires at the odd kb
                pr = kb // 2
                hid = DR_HEADS.index(h)
                for qb in qbs:
                    nc.tensor.matmul(
                        pv[h][:, qb, :], pt[:, :, qb * 128:(qb + 1) * 128],
                        v8[:, 2 * pr:2 * pr + 2, hid, :],
                        start=(pr == 0 and qb == 0),
                        stop=(pr == NPAIR - 1 and qb == 3), perf_mode=DR)
            else:
                for qb in qbs:
                    nc.tensor.matmul(
                        pv[h][:, qb, :], pt[:, qb * 128:(qb + 1) * 128],
                        v_r[:, kb, h, :],
                        start=(kb == 0 and qb == 0),
                        stop=(kb == KB - 1 and qb == 3))

        NPRE = 1
        for kb in range(KB):
            if kb == KB - 1:
                # last block: DVE-exp'd heads first so the serial DVE
                # exps (which gate the tail's reciprocals) start early
                pts = [None] * HPC
                for h in (1, 0, 2, 3):
                    pts[h] = emit_s(kb, h)
                for h in range(HPC):
                    emit_pv(prev, h, prev_pts[h])
            elif kb < NPRE:
                pts = [early_pts[(kb, h)] for h in range(HPC)]
                if prev is not None:
                    for h in range(HPC):
                        emit_pv(prev, h, prev_pts[h])
            else:
                pts = [emit_s(kb, 0), emit_s(kb, 1)]
                if prev is not None:
                    emit_pv(prev, 0, prev_pts[0])
                    emit_pv(prev, 1, prev_pts[1])
                pts += [emit_s(kb, 2), emit_s(kb, 3)]
                if prev is not None:
                    emit_pv(prev, 2, prev_pts[2])
                    emit_pv(prev, 3, prev_pts[3])
            prev, prev_pts = kb, pts
        # final key block (odd -> completes the last pair) in qb-major
        # order so the tail's per-qb normalize chains unlock in turn
        for qb in range(4):
            for h in range(HPC):
                emit_pv(prev, h, prev_pts[h], qbs=(qb,))

        # ---- tail, qb-major so each query block's normalize ->
        # transpose -> out-projection -> DMA chain drains ASAP ----
        # att[q, i] = pv[q, d] / den[q] (den = col 64 of each accumulator)
        sps_ctx.__exit__(None, None, None)  # free S banks for tps/ops
        with tc.tile_pool(name="rcp", bufs=4) as rcp, \
             tc.tile_pool(name="obuf", bufs=4) as obuf, \
             tc.tile_pool(name="tps", bufs=2, space="PSUM") as tps, \
             tc.tile_pool(name="ops", bufs=2, space="PSUM") as ops:
            Copy = mybir.ActivationFunctionType.Copy
            rcs = {}

            def recip(h):
                # one batched reciprocal per head over its 4 denominators
                rc = rcp.tile([128, 4, 1], f32, tag=f"rc{h}", name=f"rc{h}")
                nc.vector.reciprocal(rc[:], pv[h][:, :, 64:65])
                rcs[h] = rc

            def norm_mul(h, qb):
                dst = att[:, qb, h // 2, (h % 2) * 64:(h % 2 + 1) * 64]
                if h % 2 == 0:
                    nc.vector.tensor_scalar(dst, pv[h][:, qb, 0:64],
                                            rcs[h][:, qb, :], None, MUL)
                else:
                    nc.scalar.activation(dst, pv[h][:, qb, 0:64], Copy,
                                         scale=rcs[h][:, qb, :])

            for h in range(HPC):
                recip(h)
            for qb in range(4):
                for h in range(HPC):
                    norm_mul(h, qb)
                for c in range(2):
                    tp = tps.tile([128, 128], bf16, tag="tp", name="tp")
                    nc.tensor.transpose(tp[:], att[:, qb, c, :], id_s[:])
                    if c == 0:
                        nc.vector.tensor_copy(attnT[:, c, qb, :], tp[:])
                    else:
                        nc.scalar.copy(attnT[:, c, qb, :], tp[:])
                op = ops.tile([128, OUT_DIM], f32, tag="op", name="op")
                for c in range(2):
                    nc.tensor.matmul(op[:], attnT[:, c, qb, :], wo_s[:, c, :],
                                     start=(c == 0), stop=(c == 1))
                ob = obuf.tile([128, OUT_DIM], bf16, tag="ob", name="ob")
                if qb % 2 == 0:
                    nc.vector.tensor_copy(ob[:], op[:])
                else:
                    nc.scalar.copy(ob[:], op[:])
                nc.sync.dma_start(outp[:, qb, :], ob[:])


def build():
    if "nc" in _CACHE:
        return _CACHE["nc"]
    from contextlib import ExitStack

    import concourse.tile as tile
    from concourse import bacc

    nc = bacc.Bacc("TRN2", target_bir_lowering=False, debug=False,
                   num_devices=NCORES)
    with tile.TileContext(nc) as tc:
        with ExitStack() as ctx:
            _emit(ctx, tc, nc)
    nc.compile()
    _CACHE["nc"] = nc
    return nc


def _pm(a, nblk):
    """[nblk*128, f] -> partition-major [128, nblk, f] (bf16)."""
    import ml_dtypes

    f = a.shape[1]
    return np.ascontiguousarray(
        a.reshape(nblk, 128, f).transpose(1, 0, 2)).astype(ml_dtypes.bfloat16)


def _pm_hilo(a, nblk):
    """[nblk*128, f] f32 -> partition-major e4m3 (hi, lo) pair."""
    import ml_dtypes

    e4 = ml_dtypes.float8_e4m3
    f = a.shape[1]
    pm = np.ascontiguousarray(
        a.reshape(nblk, 128, f).transpose(1, 0, 2)).astype(np.float32)
    hi = pm.astype(e4)
    lo = (pm - hi.astype(np.float32)).astype(e4)
    return hi, lo


def shard(inputs):
    import ml_dtypes

    data = np.asarray(inputs["data"], dtype=np.float32)
    latent = np.asarray(inputs["latent"], dtype=np.float32)
    wq = np.asarray(inputs["Wq"], dtype=np.float32)
    wk = np.asarray(inputs["Wk"], dtype=np.float32) * WSCALE
    wv = np.asarray(inputs["Wv"], dtype=np.float32) * WSCALE
    wo = np.asarray(inputs["Wo"], dtype=np.float32)

    dataT = [_pm_hilo(np.ascontiguousarray(data[b].T), 2) for b in range(B)]
    latT = [_pm(np.ascontiguousarray(latent[b].T), 4) for b in range(B)]
    idn = np.eye(128, dtype=ml_dtypes.bfloat16)

    per_g = []
    for g in range(2):
        rows = slice(g * IH, (g + 1) * IH)
        kh, kl = _pm_hilo(np.ascontiguousarray(wk[rows, :].T), 2)
        vh, vl = _pm_hilo(np.ascontiguousarray(wv[rows, :].T), 2)
        per_g.append({
            "wqT": _pm(np.ascontiguousarray(wq[rows, :].T), 4),
            "wkhi": kh, "wklo": kl, "wvhi": vh, "wvlo": vl,
            "woT": _pm(np.ascontiguousarray(wo[:, rows].T), 2),
        })

    in_maps = []
    for i in range(NCORES):
        b, g = i // 2, i % 2
        in_maps.append({
            "dhi": dataT[b][0], "dlo": dataT[b][1],
            "latentT": latT[b], "ident": idn, **per_g[g],
        })
    return in_maps


def unshard(results, bo):
    out = np.empty((B, LS, OUT_DIM), dtype=np.float32)
    for b in range(B):
        o0 = np.asarray(results[2 * b]["outp"], dtype=np.float32)
        o1 = np.asarray(results[2 * b + 1]["outp"], dtype=np.float32)
        o = ((o0 + o1) / WSCALE).reshape(128, 4, OUT_DIM).transpose(1, 0, 2)
        out[b] = o.reshape(LS, OUT_DIM) + bo
    return out


def run(inputs, trace=False):
    from concourse import bass_utils

    nc = build()
    in_maps = shard(inputs)
    res = bass_utils.run_bass_kernel_spmd(
        nc, in_maps, core_ids=list(range(NCORES)), trace=trace)
    bo = np.asarray(inputs["bo"], dtype=np.float32).reshape(OUT_DIM)
    return unshard(res.results, bo), res


def kernel(**inputs):
    return run(inputs)[0]


# revision 29
# speedup vs baseline: 1.0107x; 1.0107x over previous
"""Trainium2 Bass kernel for the latent-query attention module.

Module math (fp32 inputs):
  Q = latent @ Wq.T; K = data @ Wk.T; V = data @ Wv.T
  S = (Q K^T)/sqrt(D); P = softmax_keys(S); out = (P V) @ Wo.T + bo

Sharding: 8 cores = 4 batches x 2 head-groups (4 heads each). Each core
computes Q/K/V for its heads, full attention over all 4096 keys and all
512 queries, and a PARTIAL output projection attn_g @ Wo[:, g].T.
Host gather sums the two partials per batch, divides by 16 (Wk/Wv ship
x16, below) and adds the bias.

Cost-model-aware design (graded time = concourse TimelineSim); this is
the baseline schedule plus two pure-PE reductions:
  - K/V projections run as fp8e4m3 DoubleRow (0.5 cycles/row AND 2x128
    contraction per instruction) using host-side hi/lo error-feedback
    splits of data and of 16x-scaled Wk/Wv (16x keeps the lo residues
    out of e4m3's subnormal range):  K ~ dhi@whi + dlo@whi + dhi@wlo.
    3 DR instructions replace 4 bf16 ones at better-than-bf16 accuracy.
  - Attention stays law-split BY HEAD (softmax purity): heads 0,3 use
    ACT true Exp emitting P directly as e4m3 (scale/16, bias -1); their
    PV runs as key-block-PAIRED fp8 DoubleRow against an e4m3 copy of
    V (+ones) that the otherwise-idle GPSIMD converts SBUF->SBUF from
    the bf16 V. 4x fewer PE cycles for those heads' PV. Heads 1,2 keep
    the DVE Schraudolph bf16 exp and the bf16 65-wide PV, so their
    softmax stays at bf16 precision (rel-err ~1.1e-2 total, tol 2e-2).
  - The ones column makes PV col 64 the softmax denominator (the 1.0 is
    not Wv-scaled, so att carries a clean 16x removed on host).
"""

import sys

sys.path.insert(0, "/opt/trn_rl_repo")

import numpy as np

B, DS, DC = 4, 4096, 256
LS, LC = 512, 512
H, D = 8, 64
INNER, OUT_DIM = 512, 512
NCORES = 8
HPC = 4                 # heads per core
IH = HPC * D            # inner half = 256
KB = DS // 128          # 32 key blocks
NPAIR = KB // 2         # 16 key-block pairs
NCH = DS // 512         # 8 data chunks
SCALE = D ** -0.5
WSCALE = 16.0           # Wk/Wv host-side scale (hi/lo fp8 subnormal fix)

# Schraudolph exp for bf16 bit patterns: bf16bits(exp(s)) ~ EA*s + EB
# (SCALE and the 1/16 K-scale folded into EA).
EA = 128.0 * 1.4426950408889634 * SCALE / WSCALE
EB = 16256.0 - 5.59 + 0.5
# ACT heads: P = exp(s*SCALE/16 - C_ACT) in e4m3 (max ~90 << 240)
C_ACT = 1.0

DR_HEADS = (0, 3)       # ACT true-exp heads -> e4m3 DoubleRow PV
_CACHE = {}


def _emit(ctx, tc, nc):
    from concourse import mybir

    f32 = mybir.dt.float32
    bf16 = mybir.dt.bfloat16
    e4 = mybir.dt.float8e4
    i16 = mybir.dt.int16
    Exp = mybir.ActivationFunctionType.Exp
    MUL = mybir.AluOpType.mult
    ADD = mybir.AluOpType.add
    DR = mybir.MatmulPerfMode.DoubleRow

    # ---- DRAM I/O (partition-major; see shard()) ----
    latentT = nc.dram_tensor("latentT", [128, 4, LS], bf16, kind="ExternalInput").ap()
    wqT = nc.dram_tensor("wqT", [128, 4, IH], bf16, kind="ExternalInput").ap()
    dpk = nc.dram_tensor("dpk", [128, 2, 2, DS], e4, kind="ExternalInput").ap()
    wk8 = nc.dram_tensor("wk8", [128, 2, 2, IH], e4, kind="ExternalInput").ap()
    wv8 = nc.dram_tensor("wv8", [128, 2, 2, IH], e4, kind="ExternalInput").ap()
    woT = nc.dram_tensor("woT", [128, 2, OUT_DIM], bf16, kind="ExternalInput").ap()
    ident = nc.dram_tensor("ident", [128, 128], bf16, kind="ExternalInput").ap()
    outp = nc.dram_tensor("outp", [128, 4, OUT_DIM], bf16,
                          kind="ExternalOutput").ap()

    # ---- resident SBUF ----
    res = ctx.enter_context(tc.tile_pool(name="res", bufs=1))
    kt = res.tile([128, 2, DS], bf16, name="kt")        # K^T head-pairs
    v_r = res.tile([128, KB, HPC, 65], bf16, name="v")  # V + ones col
    v8 = res.tile([128, KB, 2, 65], e4, name="v8")      # e4m3 V, DR heads
    qt = res.tile([128, HPC, LS], bf16, name="qt")      # Q^T zero-padded
    att = res.tile([128, 4, 2, 128], bf16, name="att")  # normalized [q, i]
    attnT = res.tile([128, 2, 4, 128], bf16, name="attnT")
    cbias = res.tile([128, 1], f32, name="cbias")
    wts = ctx.enter_context(tc.tile_pool(name="wts", bufs=1))
    lat_s = wts.tile([128, 4, LS], bf16, name="lat_s")
    wq_s = wts.tile([128, 4, IH], bf16, name="wq_s")
    wk_s = wts.tile([128, 2, 2, IH], e4, name="wk_s")
    wv_s = wts.tile([128, 2, 2, IH], e4, name="wv_s")
    wo_s = wts.tile([128, 2, OUT_DIM], bf16, name="wo_s")
    id_s = wts.tile([128, 128], bf16, name="id_s")

    # input DMAs, spread over issuing engines so chunk 0 arrives ASAP:
    # SP: the 8 data chunk pairs (chunk 0 first); Pool (SWDGE): weights;
    # ACT: latent early, wo/id behind the qt copies.
    nc.gpsimd.dma_start(wk_s[:], wk8)
    nc.gpsimd.dma_start(wq_s[:], wqT)
    nc.gpsimd.dma_start(wv_s[:], wv8)
    nc.scalar.dma_start(lat_s[:], latentT)
    nc.scalar.dma_start(wo_s[:], woT)
    nc.scalar.dma_start(id_s[:], ident)

    # ---- PE warmup: ~3us of dummy matmuls during the DMA lead-in so
    # the cost model's p-state ramp finishes before real work arrives. ----
    wu = res.tile([128, 72], bf16, name="wu")
    nc.vector.memset(wu[:], 0.0)
    nc.vector.memset(cbias[:], -C_ACT)
    with tc.tile_pool(name="wps", bufs=1, space="PSUM") as wps:
        wp = wps.tile([8, 64], f32, name="wp")
        for _ in range(60):
            nc.tensor.matmul(wp[:], wu[:, 0:8], wu[:, 8:72],
                             start=True, stop=True)
    nc.gpsimd.memset(qt[:], 0.0)
    nc.gpsimd.memset(v_r[:, :, :, 64:65], 1.0)

    # exp engine schedule per head-slot (GPSIMD cannot read PSUM on hw):
    # ACT true Exp -> e4m3 pair tiles for heads 0/3; DVE Schraudolph
    # -> bf16 for heads 1/2.
    EXP_ENG = [0, 1, 1, 0]
    ptp = ctx.enter_context(tc.tile_pool(name="ptp", bufs=3))
    pt8p = ctx.enter_context(tc.tile_pool(name="pt8p", bufs=2))
    pt8cur = {}

    def exp_kb(kb, h, s_ap):
        """Returns the P tile for (kb, h): bf16 per-kb tile for sch heads,
        e4m3 pair tile (allocated on even kb) for DR heads."""
        if EXP_ENG[h] == 0:
            if kb % 2 == 0:
                pt8cur[h] = pt8p.tile([128, 2, 512], e4, tag=f"pt8{h}",
                                      name=f"pt8{h}")
            pt = pt8cur[h]
            nc.scalar.activation(pt[:, kb % 2, :], s_ap, Exp,
                                 scale=SCALE / WSCALE, bias=cbias[:])
            return pt
        pt = ptp.tile([128, 512], bf16, tag=f"pt{h}", name=f"pt{h}")
        nc.vector.tensor_scalar(pt[:].bitcast(i16), s_ap, EA, EB, MUL, ADD)
        return pt

    early_pts = {}

    # ---- phases 0+1: Q^T after chunk 0, K^T/V streamed over 8 chunks ----
    # sEp=2: key-blocks 0..3's S+exp units (16) run in phase-1 ACT/DVE
    # slack, cutting 4 rounds off the DVE-exp-paced phase 2.
    with tc.tile_pool(name="dstage", bufs=3) as dstage, \
         tc.tile_pool(name="kvps", bufs=2, space="PSUM") as kvps, \
         tc.tile_pool(name="sEp", bufs=1, space="PSUM") as sEp, \
         tc.tile_pool(name="vps", bufs=3, space="PSUM") as vps:

        def load_chunk(ch):
            d2 = dstage.tile([128, 2, 2, 512], e4, tag="d", name="d2")
            nc.sync.dma_start(d2[:], dpk[:, :, :, ch * 512:(ch + 1) * 512])
            return d2[:, 0, :, :], d2[:, 1, :, :]

        def kv_proj(ch, dh, dl, v_first=False):
            def k_part():
                # K^T = 16*(Wk data^T): 3-term hi/lo fp8 DoubleRow
                kp = kvps.tile([128, 2, 512], f32, tag="kp", name="kp")
                for m in range(2):
                    mc = slice(m * 128, (m + 1) * 128)
                    nc.tensor.matmul(kp[:, m, :], wk_s[:, 0, :, mc], dh[:],
                                     start=True, stop=False, perf_mode=DR)
                    nc.tensor.matmul(kp[:, m, :], wk_s[:, 0, :, mc], dl[:],
                                     start=False, stop=False, perf_mode=DR)
                    nc.tensor.matmul(kp[:, m, :], wk_s[:, 1, :, mc], dh[:],
                                     start=False, stop=True, perf_mode=DR)
                nc.scalar.copy(kt[:, :, ch * 512:(ch + 1) * 512], kp[:])
            if not v_first:
                k_part()
            for k2 in range(2):
                # V = 16*(data Wv^T): per key-128-block 3-term DR
                vp = vps.tile([128, 2, IH], f32, tag="vp", name="vp")
                for i in range(2):
                    ic = slice((2 * k2 + i) * 128, (2 * k2 + i + 1) * 128)
                    nc.tensor.matmul(vp[:, i, :], dh[:, :, ic], wv_s[:, 0, :, :],
                                     start=True, stop=False, perf_mode=DR)
                    nc.tensor.matmul(vp[:, i, :], dl[:, :, ic], wv_s[:, 0, :, :],
                                     start=False, stop=False, perf_mode=DR)
                    nc.tensor.matmul(vp[:, i, :], dh[:, :, ic], wv_s[:, 1, :, :],
                                     start=False, stop=True, perf_mode=DR)
                eng = nc.scalar if (v_first and k2 == 0) else nc.vector
                dst = v_r[:, ch * 4 + 2 * k2:ch * 4 + 2 * k2 + 2, :, 0:64]
                src = vp[:].rearrange("p b (h e) -> p b h e", e=64)
                if eng is nc.scalar:
                    eng.copy(dst, src)
                else:
                    eng.tensor_copy(dst, src)
            # idle GPSIMD converts this chunk's DR-head V to e4m3 (incl
            # the ones column written by the memset above)
            nc.gpsimd.tensor_copy(v8[:, 4 * ch:4 * ch + 4, :, :],
                                  v_r[:, 4 * ch:4 * ch + 4, 0::3, :])
            if v_first:
                k_part()

        drs = [load_chunk(0), load_chunk(1), load_chunk(2)]
        kv_proj(0, *drs[0])
        kv_proj(1, *drs[1])
        drs.append(load_chunk(3))
        kv_proj(2, *drs[2])
        # Q^T projection into the zero-padded per-head copies
        qp = kvps.tile([128, 2, 512], f32, tag="kp", name="kp")
        for m in range(2):
            for c in range(4):
                nc.tensor.matmul(qp[:, m, :],
                                 wq_s[:, c, m * 128:(m + 1) * 128],
                                 lat_s[:, c, :], start=(c == 0), stop=(c == 3))
            # rows 0:64 = head 2m, rows 64:128 = head 2m+1
            nc.scalar.copy(qt[0:64, 2 * m, :], qp[0:64, m, :])
            nc.scalar.copy(qt[64:128, 2 * m + 1, :], qp[64:128, m, :])
        # prefill schedule: key-block 0's S+exp units, one per chunk
        PREFILL = {3: [(0, 0)], 4: [(0, 1)], 5: [(0, 2)], 6: [(0, 3)],
                   7: []}

        def prefill(kb, h):
            sE = sEp.tile([128, 512], f32, tag="se", name="se")
            nc.tensor.matmul(sE[:], kt[:, h // 2, kb * 128:(kb + 1) * 128],
                             qt[:, h, :], start=True, stop=True)
            early_pts[(kb, h)] = exp_kb(kb, h, sE[:])

        for ch in range(3, NCH):
            if ch + 1 < NCH:
                drs.append(load_chunk(ch + 1))
            kv_proj(ch, *drs[ch], v_first=(ch == NCH - 1))
            for kb, h in PREFILL[ch]:
                prefill(kb, h)

    # ---- phase 2: attention (S -> exp -> PV), streamed over key blocks ----
    if True:
        pvps_ctx = tc.tile_pool(name="pvps", bufs=1, space="PSUM")
        pvps = pvps_ctx.__enter__()
        sps_ctx = tc.tile_pool(name="sps", bufs=1, space="PSUM")
        sps = sps_ctx.__enter__()
        pv = [pvps.tile([128, 4, 65], f32, name=f"pv{h}") for h in range(HPC)]
        prev = None

        def emit_s(kb, h):
            s_ = sps.tile([128, 512], f32, tag=f"s{h}", name=f"s{h}")
            nc.tensor.matmul(s_[:], kt[:, h // 2, kb * 128:(kb + 1) * 128],
                             qt[:, h, :], start=True, stop=True)
            return exp_kb(kb, h, s_[:])

        def emit_pv(kb, h, pt, qbs=range(4)):
            if EXP_ENG[h] == 0:
                if kb % 2 == 0:
                    return  # pair incomplete; fires at the odd kb
                pr = kb // 2
                hid = DR_HEADS.index(h)
                for qb in qbs:
                    nc.tensor.matmul(
                        pv[h][:, qb, :], pt[:, :, qb * 128:(qb + 1) * 128],
                        v8[:, 2 * pr:2 * pr + 2, hid, :],
                        start=(pr == 0 and qb == 0),
                        stop=(pr == NPAIR - 1 and qb == 3), perf_mode=DR)
            else:
                for qb in qbs:
                    nc.tensor.matmul(
                        pv[h][:, qb, :], pt[:, qb * 128:(qb + 1) * 128],
                        v_r[:, kb, h, :],
                        start=(kb == 0 and qb == 0),
                        stop=(kb == KB - 1 and qb == 3))

        NPRE = 1
        for kb in range(KB):
            if kb == KB - 1:
                # last block: DVE-exp'd heads first so the serial DVE
                # exps (which gate the tail's reciprocals) start early
                pts = [None] * HPC
                for h in (1, 0, 2, 3):
                    pts[h] = emit_s(kb, h)
                for h in range(HPC):
                    emit_pv(prev, h, prev_pts[h])
            elif kb < NPRE:
                pts = [early_pts[(kb, h)] for h in range(HPC)]
                if prev is not None:
                    for h in range(HPC):
                        emit_pv(prev, h, prev_pts[h])
            else:
                pts = [emit_s(kb, 0), emit_s(kb, 1)]
                if prev is not None:
                    emit_pv(prev, 0, prev_pts[0])
                    emit_pv(prev, 1, prev_pts[1])
                pts += [emit_s(kb, 2), emit_s(kb, 3)]
                if prev is not None:
                    emit_pv(prev, 2, prev_pts[2])
                    emit_pv(prev, 3, prev_pts[3])
            prev, prev_pts = kb, pts
        # final key block (odd -> completes the last pair) in qb-major
        # order so the tail's per-qb normalize chains unlock in turn
        for qb in range(4):
            for h in range(HPC):
                emit_pv(prev, h, prev_pts[h], qbs=(qb,))

        # ---- tail, qb-major so each query block's normalize ->
        # transpose -> out-projection -> DMA chain drains ASAP ----
        # att[q, i] = pv[q, d] / den[q] (den = col 64 of each accumulator)
        sps_ctx.__exit__(None, None, None)  # free S banks for tps/ops
        with tc.tile_pool(name="rcp", bufs=4) as rcp, \
             tc.tile_pool(name="obuf", bufs=4) as obuf, \
             tc.tile_pool(name="tps", bufs=2, space="PSUM") as tps, \
             tc.tile_pool(name="ops", bufs=2, space="PSUM") as ops:
            Copy = mybir.ActivationFunctionType.Copy
            rcs = {}

            def recip(h):
                # one batched reciprocal per head over its 4 denominators
                rc = rcp.tile([128, 4, 1], f32, tag=f"rc{h}", name=f"rc{h}")
                nc.vector.reciprocal(rc[:], pv[h][:, :, 64:65])
                rcs[h] = rc

            def norm_mul(h, qb):
                dst = att[:, qb, h // 2, (h % 2) * 64:(h % 2 + 1) * 64]
                if h % 2 == 0:
                    nc.vector.tensor_scalar(dst, pv[h][:, qb, 0:64],
                                            rcs[h][:, qb, :], None, MUL)
                else:
                    nc.scalar.activation(dst, pv[h][:, qb, 0:64], Copy,
                                         scale=rcs[h][:, qb, :])

            for h in range(HPC):
                recip(h)
            for qb in range(4):
                for h in range(HPC):
                    norm_mul(h, qb)
                for c in range(2):
                    tp = tps.tile([128, 128], bf16, tag="tp", name="tp")
                    nc.tensor.transpose(tp[:], att[:, qb, c, :], id_s[:])
                    if c == 0:
                        nc.vector.tensor_copy(attnT[:, c, qb, :], tp[:])
                    else:
                        nc.scalar.copy(attnT[:, c, qb, :], tp[:])
                op = ops.tile([128, OUT_DIM], f32, tag="op", name="op")
                for c in range(2):
                    nc.tensor.matmul(op[:], attnT[:, c, qb, :], wo_s[:, c, :],
                                     start=(c == 0), stop=(c == 1))
                ob = obuf.tile([128, OUT_DIM], bf16, tag="ob", name="ob")
                if qb % 2 == 0:
                    nc.vector.tensor_copy(ob[:], op[:])
                else:
                    nc.scalar.copy(ob[:], op[:])
                nc.sync.dma_start(outp[:, qb, :], ob[:])


def build():
    if "nc" in _CACHE:
        return _CACHE["nc"]
    from contextlib import ExitStack

    import concourse.tile as tile
    from concourse import bacc

    nc = bacc.Bacc("TRN2", target_bir_lowering=False, debug=False,
                   num_devices=NCORES)
    with tile.TileContext(nc) as tc:
        with ExitStack() as ctx:
            _emit(ctx, tc, nc)
    nc.compile()
    _CACHE["nc"] = nc
    return nc


def _pm(a, nblk):
    """[nblk*128, f] -> partition-major [128, nblk, f] (bf16)."""
    import ml_dtypes

    f = a.shape[1]
    return np.ascontiguousarray(
        a.reshape(nblk, 128, f).transpose(1, 0, 2)).astype(ml_dtypes.bfloat16)


def _pm_hilo(a, nblk):
    """[nblk*128, f] f32 -> partition-major e4m3 (hi, lo) pair."""
    import ml_dtypes

    e4 = ml_dtypes.float8_e4m3
    f = a.shape[1]
    pm = np.ascontiguousarray(
        a.reshape(nblk, 128, f).transpose(1, 0, 2)).astype(np.float32)
    hi = pm.astype(e4)
    lo = (pm - hi.astype(np.float32)).astype(e4)
    return hi, lo


def shard(inputs):
    import ml_dtypes

    data = np.asarray(inputs["data"], dtype=np.float32)
    latent = np.asarray(inputs["latent"], dtype=np.float32)
    wq = np.asarray(inputs["Wq"], dtype=np.float32)
    wk = np.asarray(inputs["Wk"], dtype=np.float32) * WSCALE
    wv = np.asarray(inputs["Wv"], dtype=np.float32) * WSCALE
    wo = np.asarray(inputs["Wo"], dtype=np.float32)

    dataT = [np.stack(_pm_hilo(np.ascontiguousarray(data[b].T), 2), axis=1)
             for b in range(B)]
    latT = [_pm(np.ascontiguousarray(latent[b].T), 4) for b in range(B)]
    idn = np.eye(128, dtype=ml_dtypes.bfloat16)

    per_g = []
    for g in range(2):
        rows = slice(g * IH, (g + 1) * IH)
        per_g.append({
            "wqT": _pm(np.ascontiguousarray(wq[rows, :].T), 4),
            "wk8": np.stack(_pm_hilo(np.ascontiguousarray(wk[rows, :].T), 2),
                            axis=1),
            "wv8": np.stack(_pm_hilo(np.ascontiguousarray(wv[rows, :].T), 2),
                            axis=1),
            "woT": _pm(np.ascontiguousarray(wo[:, rows].T), 2),
        })

    in_maps = []
    for i in range(NCORES):
        b, g = i // 2, i % 2
        in_maps.append({
            "dpk": dataT[b], "latentT": latT[b], "ident": idn, **per_g[g],
        })
    return in_maps


def unshard(results, bo):
    out = np.empty((B, LS, OUT_DIM), dtype=np.float32)
    for b in range(B):
        o0 = np.asarray(results[2 * b]["outp"], dtype=np.float32)
        o1 = np.asarray(results[2 * b + 1]["outp"], dtype=np.float32)
        o = ((o0 + o1) / WSCALE).reshape(128, 4, OUT_DIM).transpose(1, 0, 2)
        out[b] = o.reshape(LS, OUT_DIM) + bo
    return out


def run(inputs, trace=False):
    from concourse import bass_utils

    nc = build()
    in_maps = shard(inputs)
    res = bass_utils.run_bass_kernel_spmd(
        nc, in_maps, core_ids=list(range(NCORES)), trace=trace)
    bo = np.asarray(inputs["bo"], dtype=np.float32).reshape(OUT_DIM)
    return unshard(res.results, bo), res


def kernel(**inputs):
    return run(inputs)[0]


# revision 30
# speedup vs baseline: 1.0125x; 1.0017x over previous
"""Trainium2 Bass kernel for the latent-query attention module.

Module math (fp32 inputs):
  Q = latent @ Wq.T; K = data @ Wk.T; V = data @ Wv.T
  S = (Q K^T)/sqrt(D); P = softmax_keys(S); out = (P V) @ Wo.T + bo

Sharding: 8 cores = 4 batches x 2 head-groups (4 heads each). Each core
computes Q/K/V for its heads, full attention over all 4096 keys and all
512 queries, and a PARTIAL output projection attn_g @ Wo[:, g].T.
Host gather sums the two partials per batch, divides by 16 (Wk/Wv ship
x16, below) and adds the bias.

Cost-model-aware design (graded time = concourse TimelineSim); this is
the baseline schedule plus two pure-PE reductions:
  - K/V projections run as fp8e4m3 DoubleRow (0.5 cycles/row AND 2x128
    contraction per instruction) using host-side hi/lo error-feedback
    splits of data and of 16x-scaled Wk/Wv (16x keeps the lo residues
    out of e4m3's subnormal range):  K ~ dhi@whi + dlo@whi + dhi@wlo.
    3 DR instructions replace 4 bf16 ones at better-than-bf16 accuracy.
  - Attention stays law-split BY HEAD (softmax purity): heads 0,3 use
    ACT true Exp emitting P directly as e4m3 (scale/16, bias -1); their
    PV runs as key-block-PAIRED fp8 DoubleRow against an e4m3 copy of
    V (+ones) that the otherwise-idle GPSIMD converts SBUF->SBUF from
    the bf16 V. 4x fewer PE cycles for those heads' PV. Heads 1,2 keep
    the DVE Schraudolph bf16 exp and the bf16 65-wide PV, so their
    softmax stays at bf16 precision (rel-err ~1.1e-2 total, tol 2e-2).
  - The ones column makes PV col 64 the softmax denominator (the 1.0 is
    not Wv-scaled, so att carries a clean 16x removed on host).
"""

import sys

sys.path.insert(0, "/opt/trn_rl_repo")

import numpy as np

B, DS, DC = 4, 4096, 256
LS, LC = 512, 512
H, D = 8, 64
INNER, OUT_DIM = 512, 512
NCORES = 8
HPC = 4                 # heads per core
IH = HPC * D            # inner half = 256
KB = DS // 128          # 32 key blocks
NPAIR = KB // 2         # 16 key-block pairs
NCH = DS // 512         # 8 data chunks
SCALE = D ** -0.5
WSCALE = 16.0           # Wk/Wv host-side scale (hi/lo fp8 subnormal fix)

# Schraudolph exp for bf16 bit patterns: bf16bits(exp(s)) ~ EA*s + EB
# (SCALE and the 1/16 K-scale folded into EA).
EA = 128.0 * 1.4426950408889634 * SCALE / WSCALE
EB = 16256.0 - 5.59 + 0.5
# ACT heads: P = exp(s*SCALE/16 - C_ACT) in e4m3 (max ~90 << 240)
C_ACT = 1.0

DR_HEADS = (0, 3)       # ACT true-exp heads -> e4m3 DoubleRow PV
_CACHE = {}


def _emit(ctx, tc, nc):
    from concourse import mybir

    f32 = mybir.dt.float32
    bf16 = mybir.dt.bfloat16
    e4 = mybir.dt.float8e4
    i16 = mybir.dt.int16
    Exp = mybir.ActivationFunctionType.Exp
    MUL = mybir.AluOpType.mult
    ADD = mybir.AluOpType.add
    DR = mybir.MatmulPerfMode.DoubleRow

    # ---- DRAM I/O (partition-major; see shard()) ----
    latentT = nc.dram_tensor("latentT", [128, 4, LS], bf16, kind="ExternalInput").ap()
    wqT = nc.dram_tensor("wqT", [128, 4, IH], bf16, kind="ExternalInput").ap()
    dpk = nc.dram_tensor("dpk", [128, 2, 2, DS], e4, kind="ExternalInput").ap()
    wk8 = nc.dram_tensor("wk8", [128, 2, 2, IH], e4, kind="ExternalInput").ap()
    wv8 = nc.dram_tensor("wv8", [128, 2, 2, IH], e4, kind="ExternalInput").ap()
    woT = nc.dram_tensor("woT", [128, 2, OUT_DIM], bf16, kind="ExternalInput").ap()
    ident = nc.dram_tensor("ident", [128, 128], bf16, kind="ExternalInput").ap()
    outp = nc.dram_tensor("outp", [128, 4, OUT_DIM], bf16,
                          kind="ExternalOutput").ap()

    # ---- resident SBUF ----
    res = ctx.enter_context(tc.tile_pool(name="res", bufs=1))
    kt = res.tile([128, 2, DS], bf16, name="kt")        # K^T head-pairs
    v_r = res.tile([128, KB, HPC, 65], bf16, name="v")  # V + ones col
    v8 = res.tile([128, KB, 2, 65], e4, name="v8")      # e4m3 V, DR heads
    qt = res.tile([128, HPC, LS], bf16, name="qt")      # Q^T zero-padded
    att = res.tile([128, 4, 2, 128], bf16, name="att")  # normalized [q, i]
    attnT = res.tile([128, 2, 4, 128], bf16, name="attnT")
    cbias = res.tile([128, 1], f32, name="cbias")
    wts = ctx.enter_context(tc.tile_pool(name="wts", bufs=1))
    lat_s = wts.tile([128, 4, LS], bf16, name="lat_s")
    wq_s = wts.tile([128, 4, IH], bf16, name="wq_s")
    wk_s = wts.tile([128, 2, 2, IH], e4, name="wk_s")
    wv_s = wts.tile([128, 2, 2, IH], e4, name="wv_s")
    wo_s = wts.tile([128, 2, OUT_DIM], bf16, name="wo_s")
    id_s = wts.tile([128, 128], bf16, name="id_s")

    # input DMAs, spread over issuing engines so chunk 0 arrives ASAP:
    # SP: the 8 data chunk pairs (chunk 0 first); Pool (SWDGE): weights;
    # ACT: latent early, wo/id behind the qt copies.
    nc.gpsimd.dma_start(wk_s[:], wk8)
    nc.gpsimd.dma_start(wq_s[:], wqT)
    nc.gpsimd.dma_start(wv_s[:], wv8)
    nc.scalar.dma_start(lat_s[:], latentT)
    nc.scalar.dma_start(wo_s[:], woT)
    nc.scalar.dma_start(id_s[:], ident)

    # ---- PE warmup: ~3us of dummy matmuls during the DMA lead-in so
    # the cost model's p-state ramp finishes before real work arrives. ----
    wu = res.tile([128, 72], bf16, name="wu")
    nc.vector.memset(wu[:], 0.0)
    nc.vector.memset(cbias[:], -C_ACT)
    with tc.tile_pool(name="wps", bufs=1, space="PSUM") as wps:
        wp = wps.tile([8, 64], f32, name="wp")
        for _ in range(60):
            nc.tensor.matmul(wp[:], wu[:, 0:8], wu[:, 8:72],
                             start=True, stop=True)
    nc.gpsimd.memset(qt[:], 0.0)
    nc.gpsimd.memset(v_r[:, :, :, 64:65], 1.0)

    # exp engine schedule per head-slot (GPSIMD cannot read PSUM on hw):
    # ACT true Exp -> e4m3 pair tiles for heads 0/3; DVE Schraudolph
    # -> bf16 for heads 1/2.
    EXP_ENG = [0, 1, 1, 0]
    ptp = ctx.enter_context(tc.tile_pool(name="ptp", bufs=3))
    pt8p = ctx.enter_context(tc.tile_pool(name="pt8p", bufs=2))
    pt8cur = {}

    def exp_kb(kb, h, s_ap):
        """Returns the P tile for (kb, h): bf16 per-kb tile for sch heads,
        e4m3 pair tile (allocated on even kb) for DR heads."""
        if EXP_ENG[h] == 0:
            if kb % 2 == 0:
                pt8cur[h] = pt8p.tile([128, 2, 512], e4, tag=f"pt8{h}",
                                      name=f"pt8{h}")
            pt = pt8cur[h]
            nc.scalar.activation(pt[:, kb % 2, :], s_ap, Exp,
                                 scale=SCALE / WSCALE, bias=cbias[:])
            return pt
        pt = ptp.tile([128, 512], bf16, tag=f"pt{h}", name=f"pt{h}")
        nc.vector.tensor_scalar(pt[:].bitcast(i16), s_ap, EA, EB, MUL, ADD)
        return pt

    early_pts = {}

    # ---- phases 0+1: Q^T after chunk 0, K^T/V streamed over 8 chunks ----
    # sEp=2: key-blocks 0..3's S+exp units (16) run in phase-1 ACT/DVE
    # slack, cutting 4 rounds off the DVE-exp-paced phase 2.
    with tc.tile_pool(name="dstage", bufs=4) as dstage, \
         tc.tile_pool(name="kvps", bufs=2, space="PSUM") as kvps, \
         tc.tile_pool(name="sEp", bufs=1, space="PSUM") as sEp, \
         tc.tile_pool(name="vps", bufs=3, space="PSUM") as vps:

        def load_chunk(ch):
            d2 = dstage.tile([128, 2, 2, 512], e4, tag="d", name="d2")
            nc.sync.dma_start(d2[:], dpk[:, :, :, ch * 512:(ch + 1) * 512])
            return d2[:, 0, :, :], d2[:, 1, :, :]

        def kv_proj(ch, dh, dl, v_first=False):
            def k_part():
                # K^T = 16*(Wk data^T): 3-term hi/lo fp8 DoubleRow
                kp = kvps.tile([128, 2, 512], f32, tag="kp", name="kp")
                for m in range(2):
                    mc = slice(m * 128, (m + 1) * 128)
                    nc.tensor.matmul(kp[:, m, :], wk_s[:, 0, :, mc], dh[:],
                                     start=True, stop=False, perf_mode=DR)
                    nc.tensor.matmul(kp[:, m, :], wk_s[:, 0, :, mc], dl[:],
                                     start=False, stop=False, perf_mode=DR)
                    nc.tensor.matmul(kp[:, m, :], wk_s[:, 1, :, mc], dh[:],
                                     start=False, stop=True, perf_mode=DR)
                nc.scalar.copy(kt[:, :, ch * 512:(ch + 1) * 512], kp[:])
            if not v_first:
                k_part()
            for k2 in range(2):
                # V = 16*(data Wv^T): per key-128-block 3-term DR
                vp = vps.tile([128, 2, IH], f32, tag="vp", name="vp")
                for i in range(2):
                    ic = slice((2 * k2 + i) * 128, (2 * k2 + i + 1) * 128)
                    nc.tensor.matmul(vp[:, i, :], dh[:, :, ic], wv_s[:, 0, :, :],
                                     start=True, stop=False, perf_mode=DR)
                    nc.tensor.matmul(vp[:, i, :], dl[:, :, ic], wv_s[:, 0, :, :],
                                     start=False, stop=False, perf_mode=DR)
                    nc.tensor.matmul(vp[:, i, :], dh[:, :, ic], wv_s[:, 1, :, :],
                                     start=False, stop=True, perf_mode=DR)
                eng = nc.scalar if (v_first and k2 == 0) else nc.vector
                dst = v_r[:, ch * 4 + 2 * k2:ch * 4 + 2 * k2 + 2, :, 0:64]
                src = vp[:].rearrange("p b (h e) -> p b h e", e=64)
                if eng is nc.scalar:
                    eng.copy(dst, src)
                else:
                    eng.tensor_copy(dst, src)
            # idle GPSIMD converts this chunk's DR-head V to e4m3 (incl
            # the ones column written by the memset above)
            nc.gpsimd.tensor_copy(v8[:, 4 * ch:4 * ch + 4, :, :],
                                  v_r[:, 4 * ch:4 * ch + 4, 0::3, :])
            if v_first:
                k_part()

        drs = [load_chunk(0), load_chunk(1), load_chunk(2)]
        kv_proj(0, *drs[0])
        kv_proj(1, *drs[1])
        drs.append(load_chunk(3))
        kv_proj(2, *drs[2])
        # Q^T projection into the zero-padded per-head copies
        qp = kvps.tile([128, 2, 512], f32, tag="kp", name="kp")
        for m in range(2):
            for c in range(4):
                nc.tensor.matmul(qp[:, m, :],
                                 wq_s[:, c, m * 128:(m + 1) * 128],
                                 lat_s[:, c, :], start=(c == 0), stop=(c == 3))
            # rows 0:64 = head 2m, rows 64:128 = head 2m+1
            nc.scalar.copy(qt[0:64, 2 * m, :], qp[0:64, m, :])
            nc.scalar.copy(qt[64:128, 2 * m + 1, :], qp[64:128, m, :])
        # prefill schedule: key-block 0's S+exp units, one per chunk
        PREFILL = {3: [(0, 0)], 4: [(0, 1)], 5: [(0, 2)], 6: [(0, 3)],
                   7: []}

        def prefill(kb, h):
            sE = sEp.tile([128, 512], f32, tag="se", name="se")
            nc.tensor.matmul(sE[:], kt[:, h // 2, kb * 128:(kb + 1) * 128],
                             qt[:, h, :], start=True, stop=True)
            early_pts[(kb, h)] = exp_kb(kb, h, sE[:])

        for ch in range(3, NCH):
            if ch + 1 < NCH:
                drs.append(load_chunk(ch + 1))
            kv_proj(ch, *drs[ch], v_first=(ch == NCH - 1))
            for kb, h in PREFILL[ch]:
                prefill(kb, h)

    # ---- phase 2: attention (S -> exp -> PV), streamed over key blocks ----
    if True:
        pvps_ctx = tc.tile_pool(name="pvps", bufs=1, space="PSUM")
        pvps = pvps_ctx.__enter__()
        sps_ctx = tc.tile_pool(name="sps", bufs=1, space="PSUM")
        sps = sps_ctx.__enter__()
        pv = [pvps.tile([128, 4, 65], f32, name=f"pv{h}") for h in range(HPC)]
        prev = None

        def emit_s(kb, h):
            s_ = sps.tile([128, 512], f32, tag=f"s{h}", name=f"s{h}")
            nc.tensor.matmul(s_[:], kt[:, h // 2, kb * 128:(kb + 1) * 128],
                             qt[:, h, :], start=True, stop=True)
            return exp_kb(kb, h, s_[:])

        def emit_pv(kb, h, pt, qbs=range(4)):
            if EXP_ENG[h] == 0:
                if kb % 2 == 0:
                    return  # pair incomplete; fires at the odd kb
                pr = kb // 2
                hid = DR_HEADS.index(h)
                for qb in qbs:
                    nc.tensor.matmul(
                        pv[h][:, qb, :], pt[:, :, qb * 128:(qb + 1) * 128],
                        v8[:, 2 * pr:2 * pr + 2, hid, :],
                        start=(pr == 0 and qb == 0),
                        stop=(pr == NPAIR - 1 and qb == 3), perf_mode=DR)
            else:
                for qb in qbs:
                    nc.tensor.matmul(
                        pv[h][:, qb, :], pt[:, qb * 128:(qb + 1) * 128],
                        v_r[:, kb, h, :],
                        start=(kb == 0 and qb == 0),
                        stop=(kb == KB - 1 and qb == 3))

        NPRE = 1
        for kb in range(KB):
            if kb == KB - 1:
                # last block: DVE-exp'd heads first so the serial DVE
                # exps (which gate the tail's reciprocals) start early
                pts = [None] * HPC
                for h in (1, 0, 2, 3):
                    pts[h] = emit_s(kb, h)
                for h in range(HPC):
                    emit_pv(prev, h, prev_pts[h])
            elif kb < NPRE:
                pts = [early_pts[(kb, h)] for h in range(HPC)]
                if prev is not None:
                    for h in range(HPC):
                        emit_pv(prev, h, prev_pts[h])
            else:
                pts = [emit_s(kb, 0), emit_s(kb, 1)]
                if prev is not None:
                    emit_pv(prev, 0, prev_pts[0])
                    emit_pv(prev, 1, prev_pts[1])
                pts += [emit_s(kb, 2), emit_s(kb, 3)]
                if prev is not None:
                    emit_pv(prev, 2, prev_pts[2])
                    emit_pv(prev, 3, prev_pts[3])
            prev, prev_pts = kb, pts
        # final key block (odd -> completes the last pair) in qb-major
        # order so the tail's per-qb normalize chains unlock in turn
        for qb in range(4):
            for h in range(HPC):
                emit_pv(prev, h, prev_pts[h], qbs=(qb,))

        # ---- tail, qb-major so each query block's normalize ->
        # transpose -> out-projection -> DMA chain drains ASAP ----
        # att[q, i] = pv[q, d] / den[q] (den = col 64 of each accumulator)
        sps_ctx.__exit__(None, None, None)  # free S banks for tps/ops
        with tc.tile_pool(name="rcp", bufs=4) as rcp, \
             tc.tile_pool(name="obuf", bufs=4) as obuf, \
             tc.tile_pool(name="tps", bufs=2, space="PSUM") as tps, \
             tc.tile_pool(name="ops", bufs=2, space="PSUM") as ops:
            Copy = mybir.ActivationFunctionType.Copy
            rcs = {}

            def recip(h):
                # one batched reciprocal per head over its 4 denominators
                rc = rcp.tile([128, 4, 1], f32, tag=f"rc{h}", name=f"rc{h}")
                nc.vector.reciprocal(rc[:], pv[h][:, :, 64:65])
                rcs[h] = rc

            def norm_mul(h, qb):
                dst = att[:, qb, h // 2, (h % 2) * 64:(h % 2 + 1) * 64]
                if h % 2 == 0:
                    nc.vector.tensor_scalar(dst, pv[h][:, qb, 0:64],
                                            rcs[h][:, qb, :], None, MUL)
                else:
                    nc.scalar.activation(dst, pv[h][:, qb, 0:64], Copy,
                                         scale=rcs[h][:, qb, :])

            for h in range(HPC):
                recip(h)
            for qb in range(4):
                for h in range(HPC):
                    norm_mul(h, qb)
                for c in range(2):
                    tp = tps.tile([128, 128], bf16, tag="tp", name="tp")
                    nc.tensor.transpose(tp[:], att[:, qb, c, :], id_s[:])
                    if c == 0:
                        nc.vector.tensor_copy(attnT[:, c, qb, :], tp[:])
                    else:
                        nc.scalar.copy(attnT[:, c, qb, :], tp[:])
                op = ops.tile([128, OUT_DIM], f32, tag="op", name="op")
                for c in range(2):
                    nc.tensor.matmul(op[:], attnT[:, c, qb, :], wo_s[:, c, :],
                                     start=(c == 0), stop=(c == 1))
                ob = obuf.tile([128, OUT_DIM], bf16, tag="ob", name="ob")
                if qb % 2 == 0:
                    nc.vector.tensor_copy(ob[:], op[:])
                else:
                    nc.scalar.copy(ob[:], op[:])
                nc.sync.dma_start(outp[:, qb, :], ob[:])


def build():
    if "nc" in _CACHE:
        return _CACHE["nc"]
    from contextlib import ExitStack

    import concourse.tile as tile
    from concourse import bacc

    nc = bacc.Bacc("TRN2", target_bir_lowering=False, debug=False,
                   num_devices=NCORES)
    with tile.TileContext(nc) as tc:
        with ExitStack() as ctx:
            _emit(ctx, tc, nc)
    nc.compile()
    _CACHE["nc"] = nc
    return nc


def _pm(a, nblk):
    """[nblk*128, f] -> partition-major [128, nblk, f] (bf16)."""
    import ml_dtypes

    f = a.shape[1]
    return np.ascontiguousarray(
        a.reshape(nblk, 128, f).transpose(1, 0, 2)).astype(ml_dtypes.bfloat16)


def _pm_hilo(a, nblk):
    """[nblk*128, f] f32 -> partition-major e4m3 (hi, lo) pair."""
    import ml_dtypes

    e4 = ml_dtypes.float8_e4m3
    f = a.shape[1]
    pm = np.ascontiguousarray(
        a.reshape(nblk, 128, f).transpose(1, 0, 2)).astype(np.float32)
    hi = pm.astype(e4)
    lo = (pm - hi.astype(np.float32)).astype(e4)
    return hi, lo


def shard(inputs):
    import ml_dtypes

    data = np.asarray(inputs["data"], dtype=np.float32)
    latent = np.asarray(inputs["latent"], dtype=np.float32)
    wq = np.asarray(inputs["Wq"], dtype=np.float32)
    wk = np.asarray(inputs["Wk"], dtype=np.float32) * WSCALE
    wv = np.asarray(inputs["Wv"], dtype=np.float32) * WSCALE
    wo = np.asarray(inputs["Wo"], dtype=np.float32)

    dataT = [np.stack(_pm_hilo(np.ascontiguousarray(data[b].T), 2), axis=1)
             for b in range(B)]
    latT = [_pm(np.ascontiguousarray(latent[b].T), 4) for b in range(B)]
    idn = np.eye(128, dtype=ml_dtypes.bfloat16)

    per_g = []
    for g in range(2):
        rows = slice(g * IH, (g + 1) * IH)
        per_g.append({
            "wqT": _pm(np.ascontiguousarray(wq[rows, :].T), 4),
            "wk8": np.stack(_pm_hilo(np.ascontiguousarray(wk[rows, :].T), 2),
                            axis=1),
            "wv8": np.stack(_pm_hilo(np.ascontiguousarray(wv[rows, :].T), 2),
                            axis=1),
            "woT": _pm(np.ascontiguousarray(wo[:, rows].T), 2),
        })

    in_maps = []
    for i in range(NCORES):
        b, g = i // 2, i % 2
        in_maps.append({
            "dpk": dataT[b], "latentT": latT[b], "ident": idn, **per_g[g],
        })
    return in_maps


def unshard(results, bo):
    out = np.empty((B, LS, OUT_DIM), dtype=np.float32)
    for b in range(B):
        o0 = np.asarray(results[2 * b]["outp"], dtype=np.float32)
        o1 = np.asarray(results[2 * b + 1]["outp"], dtype=np.float32)
        o = ((o0 + o1) / WSCALE).reshape(128, 4, OUT_DIM).transpose(1, 0, 2)
        out[b] = o.reshape(LS, OUT_DIM) + bo
    return out


def run(inputs, trace=False):
    from concourse import bass_utils

    nc = build()
    in_maps = shard(inputs)
    res = bass_utils.run_bass_kernel_spmd(
        nc, in_maps, core_ids=list(range(NCORES)), trace=trace)
    bo = np.asarray(inputs["bo"], dtype=np.float32).reshape(OUT_DIM)
    return unshard(res.results, bo), res


def kernel(**inputs):
    return run(inputs)[0]


# revision 31
# speedup vs baseline: 1.0227x; 1.0102x over previous
"""Trainium2 Bass kernel for the latent-query attention module.

Module math (fp32 inputs):
  Q = latent @ Wq.T; K = data @ Wk.T; V = data @ Wv.T
  S = (Q K^T)/sqrt(D); P = softmax_keys(S); out = (P V) @ Wo.T + bo

Sharding: 8 cores = 4 batches x 2 head-groups (4 heads each). Each core
computes Q/K/V for its heads, full attention over all 4096 keys and all
512 queries, and a PARTIAL output projection attn_g @ Wo[:, g].T.
Host gather sums the two partials per batch and adds the bias (the
tensor-parallel all-reduce, done on host).

Cost-model-aware design (graded time = concourse TimelineSim):
  - matmul cost = out free-size N x 0.417ns (bf16/f32r); stationary
    operand (LDWEIGHTS) is free. So PV uses P^T blocks as the STATIONARY
    operand and [V_h | ones] as moving (N=65): 33k cycles instead of 66k.
    The ones column makes col 64 of each PV accumulator the softmax
    denominator, on the same partition as its queries -> normalize is a
    per-partition reciprocal + tensor_scalar multiply (no PE broadcast).
  - All inputs pre-converted to bf16 on host (rel-err ~2e-3, tol 2e-2);
    DMA'd directly, no on-device rounding passes.
  - exp over the 8.4M logits/core is the 2nd-largest engine load; it is
    split over ACT (true Exp activation) and DVE+GPSIMD (Schraudolph:
    bf16 bitpattern = int16(128*log2e*s/8 + B), one tensor_scalar).
  - K^T is stored head-pair-packed [128, 2, 4096]; Q^T zero-padded per
    head so every S matmul is a full K=128, offset-0 matmul.
  - PSUM: 4 banks S (per-head rotation) + 4 banks PV accumulators.
"""

import sys

sys.path.insert(0, "/opt/trn_rl_repo")

import numpy as np

B, DS, DC = 4, 4096, 256
LS, LC = 512, 512
H, D = 8, 64
INNER, OUT_DIM = 512, 512
NCORES = 8
HPC = 4                 # heads per core
IH = HPC * D            # inner half = 256
KB = DS // 128          # 32 key blocks
NCH = DS // 512         # 8 data chunks
SCALE = D ** -0.5

# Schraudolph exp for bf16 bit patterns: bf16bits(exp(s)) ~ EA*s + EB
# EA = 128*log2(e)*SCALE (logit scale folded in); EB = 127*128 - 5.59
# (max-rel-err-minimizing spline offset) + 0.5 (int conversion truncates
# in CoreSim; +0.5 makes truncation behave like rounding).
EA = 128.0 * 1.4426950408889634 * SCALE
EB = 16256.0 - 5.59 + 0.5

_CACHE = {}


def _emit(ctx, tc, nc):
    from concourse import mybir

    f32 = mybir.dt.float32
    bf16 = mybir.dt.bfloat16
    i16 = mybir.dt.int16
    Exp = mybir.ActivationFunctionType.Exp
    MUL = mybir.AluOpType.mult
    ADD = mybir.AluOpType.add

    # ---- DRAM I/O (bf16, partition-major; see shard()) ----
    latentT = nc.dram_tensor("latentT", [128, 4, LS], bf16, kind="ExternalInput").ap()
    wqT = nc.dram_tensor("wqT", [128, 4, IH], bf16, kind="ExternalInput").ap()
    dataT = nc.dram_tensor("dataT", [128, 2, DS], bf16, kind="ExternalInput").ap()
    wkT = nc.dram_tensor("wkT", [128, 2, IH], bf16, kind="ExternalInput").ap()
    wvT = nc.dram_tensor("wvT", [128, 2, IH], bf16, kind="ExternalInput").ap()
    woT = nc.dram_tensor("woT", [128, 2, OUT_DIM], bf16, kind="ExternalInput").ap()
    ident = nc.dram_tensor("ident", [128, 128], bf16, kind="ExternalInput").ap()
    outp = nc.dram_tensor("outp", [128, 4, OUT_DIM], bf16,
                          kind="ExternalOutput").ap()

    # ---- resident SBUF ----
    res = ctx.enter_context(tc.tile_pool(name="res", bufs=1))
    kt = res.tile([128, 2, DS], bf16, name="kt")        # K^T head-pairs
    v_r = res.tile([128, KB, HPC, 65], bf16, name="v")  # V + ones col
    qt = res.tile([128, HPC, LS], bf16, name="qt")      # Q^T zero-padded
    att = res.tile([128, 4, 2, 128], bf16, name="att")  # normalized [q, i]
    attnT = res.tile([128, 2, 4, 128], bf16, name="attnT")
    wts = ctx.enter_context(tc.tile_pool(name="wts", bufs=1))
    lat_s = wts.tile([128, 4, LS], bf16, name="lat_s")
    wq_s = wts.tile([128, 4, IH], bf16, name="wq_s")
    wk_s = wts.tile([128, 2, IH], bf16, name="wk_s")
    wv_s = wts.tile([128, 2, IH], bf16, name="wv_s")
    wo_s = wts.tile([128, 2, OUT_DIM], bf16, name="wo_s")
    id_s = wts.tile([128, 128], bf16, name="id_s")

    # input DMAs, spread over issuing engines so chunk 0 arrives ASAP
    # and nothing queues behind the phase-1 PSUM->SBUF copies:
    # SP: the 8 data chunks (chunk 0 first); Pool (SWDGE): wk, wq, latent
    # (early, Pool is otherwise idle); ACT: wv, wo, id (needed later).
    nc.gpsimd.dma_start(wk_s[:], wkT)
    nc.gpsimd.dma_start(wq_s[:], wqT)
    nc.scalar.dma_start(wv_s[:], wvT)
    nc.scalar.dma_start(lat_s[:], latentT)
    nc.scalar.dma_start(wo_s[:], woT)
    nc.scalar.dma_start(id_s[:], ident)

    # ---- PE warmup: ~3us of dummy matmuls during the DMA lead-in so
    # the cost model's p-state ramp finishes before real work arrives.
    # Tiles live in the resident pool: reusing their SBUF would serialize
    # the first data-chunk DMA behind the warmup. ----
    wu = res.tile([128, 72], bf16, name="wu")
    nc.vector.memset(wu[:], 0.0)
    with tc.tile_pool(name="wps", bufs=1, space="PSUM") as wps:
        wp = wps.tile([8, 64], f32, name="wp")
        for _ in range(60):
            nc.tensor.matmul(wp[:], wu[:, 0:8], wu[:, 8:72],
                             start=True, stop=True)
    nc.gpsimd.memset(qt[:], 0.0)
    nc.gpsimd.memset(v_r[:, :, :, 64:65], 1.0)

    # exp engine schedule per head-slot (GPSIMD cannot read PSUM on hw):
    # ACT true Exp for heads 0/3, DVE Schraudolph for heads 1/2.
    def exp_op(eng, pt_ap, s_ap):
        if eng == 0:
            nc.scalar.activation(pt_ap, s_ap, Exp, scale=SCALE)
        else:
            nc.vector.tensor_scalar(pt_ap.bitcast(i16), s_ap, EA, EB, MUL, ADD)

    EXP_ENG = [0, 1, 1, 0]
    ptp = ctx.enter_context(tc.tile_pool(name="ptp", bufs=3))
    early_pts = []

    # ---- phases 0+1: Q^T after chunk 0, K^T/V streamed over 8 chunks ----
    # vps=3 + the sEp bank: key-block 0's S+exp units run in phase-1 PE
    # slack (one per chunk), shaving ~0.9us off the PE-bound phase 2.
    with tc.tile_pool(name="dstage", bufs=3) as dstage, \
         tc.tile_pool(name="kvps", bufs=2, space="PSUM") as kvps, \
         tc.tile_pool(name="sEp", bufs=1, space="PSUM") as sEp, \
         tc.tile_pool(name="vps", bufs=3, space="PSUM") as vps:

        def load_chunk(ch):
            d_ = dstage.tile([128, 2, 512], bf16, tag="d", name="d_")
            nc.sync.dma_start(d_[:], dataT[:, :, ch * 512:(ch + 1) * 512])
            return d_

        def kv_proj(ch, d_, v_first=False):
            def k_part():
                kp = kvps.tile([128, 2, 512], f32, tag="kp", name="kp")
                for m in range(2):
                    for c in range(2):
                        nc.tensor.matmul(kp[:, m, :],
                                         wk_s[:, c, m * 128:(m + 1) * 128],
                                         d_[:, c, :], start=(c == 0),
                                         stop=(c == 1))
                nc.scalar.copy(kt[:, :, ch * 512:(ch + 1) * 512], kp[:])
            if not v_first:
                k_part()
            for k2 in range(2):
                vp = vps.tile([128, 2, IH], f32, tag="vp", name="vp")
                for i in range(2):
                    k4 = 2 * k2 + i
                    for c in range(2):
                        nc.tensor.matmul(
                            vp[:, i, :], d_[:, c, k4 * 128:(k4 + 1) * 128],
                            wv_s[:, c, :], start=(c == 0), stop=(c == 1))
                eng = nc.scalar if (v_first and k2 == 0) else nc.vector
                dst = v_r[:, ch * 4 + 2 * k2:ch * 4 + 2 * k2 + 2, :, 0:64]
                src = vp[:].rearrange("p b (h e) -> p b h e", e=64)
                if eng is nc.scalar:
                    eng.copy(dst, src)
                else:
                    eng.tensor_copy(dst, src)
            if v_first:
                k_part()

        drs = [load_chunk(0), load_chunk(1), load_chunk(2)]
        kv_proj(0, drs[0])
        kv_proj(1, drs[1])
        drs.append(load_chunk(3))
        kv_proj(2, drs[2])
        # Q^T projection into the zero-padded per-head copies (PSUM via
        # the kvps ring; pairs m=0,1 use the two banks of one kp tile)
        qp = kvps.tile([128, 2, 512], f32, tag="kp", name="kp")
        for m in range(2):
            for c in range(4):
                nc.tensor.matmul(qp[:, m, :],
                                 wq_s[:, c, m * 128:(m + 1) * 128],
                                 lat_s[:, c, :], start=(c == 0), stop=(c == 3))
            # rows 0:64 = head 2m, rows 64:128 = head 2m+1
            nc.scalar.copy(qt[0:64, 2 * m, :], qp[0:64, m, :])
            nc.scalar.copy(qt[64:128, 2 * m + 1, :], qp[64:128, m, :])
        for ch in range(3, NCH):
            if ch + 1 < NCH:
                drs.append(load_chunk(ch + 1))
            kv_proj(ch, drs[ch], v_first=(ch == NCH - 1))
            if 3 <= ch <= 6:
                h = ch - 3
                sE = sEp.tile([128, 512], f32, tag="se", name="se")
                nc.tensor.matmul(sE[:], kt[:, h // 2, 0:128], qt[:, h, :],
                                 start=True, stop=True)
                pt = ptp.tile([128, 512], bf16, tag=f"pt{h}", name=f"pt{h}")
                exp_op(EXP_ENG[h], pt[:], sE[:])
                early_pts.append(pt)

    # ---- phase 2: attention (S -> exp -> PV), streamed over key blocks ----
    if True:
        pvps_ctx = tc.tile_pool(name="pvps", bufs=1, space="PSUM")
        pvps = pvps_ctx.__enter__()
        sps_ctx = tc.tile_pool(name="sps", bufs=1, space="PSUM")
        sps = sps_ctx.__enter__()
        pv = [pvps.tile([128, 4, 65], f32, name=f"pv{h}") for h in range(HPC)]
        prev = None

        def emit_s(kb, h):
            m = h // 2
            s_ = sps.tile([128, 512], f32, tag=f"s{h}", name=f"s{h}")
            nc.tensor.matmul(s_[:], kt[:, m, kb * 128:(kb + 1) * 128],
                             qt[:, h, :], start=True, stop=True)
            pt = ptp.tile([128, 512], bf16, tag=f"pt{h}", name=f"pt{h}")
            exp_op(EXP_ENG[h], pt[:], s_[:])
            return pt

        def emit_pv(kb, h, pt):
            for qb in range(4):
                nc.tensor.matmul(
                    pv[h][:, qb, :], pt[:, qb * 128:(qb + 1) * 128],
                    v_r[:, kb, h, :],
                    start=(kb == 0 and qb == 0),
                    stop=(kb == KB - 1 and qb == 3))

        for kb in range(KB):
            if kb == KB - 1:
                # last block: DVE-exp'd heads first so the serial DVE
                # exps (which gate the tail's reciprocals) start early
                pts = [None] * HPC
                for h in (1, 0, 2, 3):
                    pts[h] = emit_s(kb, h)
                for h in range(HPC):
                    emit_pv(prev, h, prev_pts[h])
            elif kb == 0:
                pts = early_pts  # S+exp prebuilt during phase 1
            else:
                pts = [emit_s(kb, 0), emit_s(kb, 1)]
                if prev is not None:
                    emit_pv(prev, 0, prev_pts[0])
                    emit_pv(prev, 1, prev_pts[1])
                pts += [emit_s(kb, 2), emit_s(kb, 3)]
                if prev is not None:
                    emit_pv(prev, 2, prev_pts[2])
                    emit_pv(prev, 3, prev_pts[3])
            prev, prev_pts = kb, pts
        # final key block in qb-major order so the tail's per-qb
        # normalize chains unlock one query block at a time
        for qb in range(4):
            for h in range(HPC):
                nc.tensor.matmul(
                    pv[h][:, qb, :],
                    prev_pts[h][:, qb * 128:(qb + 1) * 128],
                    v_r[:, prev, h, :],
                    start=False, stop=(qb == 3))

        # ---- tail, qb-major so each query block's normalize ->
        # transpose -> out-projection -> DMA chain drains ASAP ----
        # att[q, i] = pv[q, d] / den[q] (den = col 64 of each accumulator)
        sps_ctx.__exit__(None, None, None)  # free S banks for tps/ops
        with tc.tile_pool(name="rcp", bufs=4) as rcp, \
             tc.tile_pool(name="obuf", bufs=4) as obuf, \
             tc.tile_pool(name="tps", bufs=2, space="PSUM") as tps, \
             tc.tile_pool(name="ops", bufs=2, space="PSUM") as ops:
            Copy = mybir.ActivationFunctionType.Copy
            rcs = {}

            def recip(h):
                # one batched reciprocal per head over its 4 denominators
                rc = rcp.tile([128, 4, 1], f32, tag=f"rc{h}", name=f"rc{h}")
                nc.vector.reciprocal(rc[:], pv[h][:, :, 64:65])
                rcs[h] = rc

            def norm_mul(h, qb):
                dst = att[:, qb, h // 2, (h % 2) * 64:(h % 2 + 1) * 64]
                if h % 2 == 0:
                    nc.vector.tensor_scalar(dst, pv[h][:, qb, 0:64],
                                            rcs[h][:, qb, :], None, MUL)
                else:
                    nc.scalar.activation(dst, pv[h][:, qb, 0:64], Copy,
                                         scale=rcs[h][:, qb, :])

            for h in range(HPC):
                recip(h)
            for qb in range(4):
                for h in range(HPC):
                    norm_mul(h, qb)
                for c in range(2):
                    tp = tps.tile([128, 128], bf16, tag="tp", name="tp")
                    nc.tensor.transpose(tp[:], att[:, qb, c, :], id_s[:])
                    if c == 0:
                        nc.vector.tensor_copy(attnT[:, c, qb, :], tp[:])
                    else:
                        nc.scalar.copy(attnT[:, c, qb, :], tp[:])
                op = ops.tile([128, OUT_DIM], f32, tag="op", name="op")
                for c in range(2):
                    nc.tensor.matmul(op[:], attnT[:, c, qb, :], wo_s[:, c, :],
                                     start=(c == 0), stop=(c == 1))
                ob = obuf.tile([128, OUT_DIM], bf16, tag="ob", name="ob")
                if qb % 2 == 0:
                    nc.vector.tensor_copy(ob[:], op[:])
                else:
                    nc.scalar.copy(ob[:], op[:])
                nc.sync.dma_start(outp[:, qb, :], ob[:])


def build():
    if "nc" in _CACHE:
        return _CACHE["nc"]
    from contextlib import ExitStack

    import concourse.tile as tile
    from concourse import bacc

    nc = bacc.Bacc("TRN2", target_bir_lowering=False, debug=False,
                   num_devices=NCORES)
    with tile.TileContext(nc) as tc:
        with ExitStack() as ctx:
            _emit(ctx, tc, nc)
    nc.compile()
    _CACHE["nc"] = nc
    return nc


def _pm(a, nblk):
    """[nblk*128, f] -> partition-major [128, nblk, f] (bf16)."""
    import ml_dtypes

    f = a.shape[1]
    return np.ascontiguousarray(
        a.reshape(nblk, 128, f).transpose(1, 0, 2)).astype(ml_dtypes.bfloat16)


def shard(inputs):
    import ml_dtypes

    data = np.asarray(inputs["data"], dtype=np.float32)
    latent = np.asarray(inputs["latent"], dtype=np.float32)
    wq = np.asarray(inputs["Wq"], dtype=np.float32)
    wk = np.asarray(inputs["Wk"], dtype=np.float32)
    wv = np.asarray(inputs["Wv"], dtype=np.float32)
    wo = np.asarray(inputs["Wo"], dtype=np.float32)

    dataT = [_pm(np.ascontiguousarray(data[b].T), 2) for b in range(B)]
    latT = [_pm(np.ascontiguousarray(latent[b].T), 4) for b in range(B)]
    idn = np.eye(128, dtype=ml_dtypes.bfloat16)

    per_g = []
    for g in range(2):
        rows = slice(g * IH, (g + 1) * IH)
        per_g.append({
            "wqT": _pm(np.ascontiguousarray(wq[rows, :].T), 4),
            "wkT": _pm(np.ascontiguousarray(wk[rows, :].T), 2),
            "wvT": _pm(np.ascontiguousarray(wv[rows, :].T), 2),
            "woT": _pm(np.ascontiguousarray(wo[:, rows].T), 2),
        })

    in_maps = []
    for i in range(NCORES):
        b, g = i // 2, i % 2
        in_maps.append({
            "dataT": dataT[b], "latentT": latT[b], "ident": idn, **per_g[g],
        })
    return in_maps


def unshard(results, bo):
    out = np.empty((B, LS, OUT_DIM), dtype=np.float32)
    for b in range(B):
        o0 = np.asarray(results[2 * b]["outp"], dtype=np.float32)
        o1 = np.asarray(results[2 * b + 1]["outp"], dtype=np.float32)
        o = (o0 + o1).reshape(128, 4, OUT_DIM).transpose(1, 0, 2)
        out[b] = o.reshape(LS, OUT_DIM) + bo
    return out


def run(inputs, trace=False):
    from concourse import bass_utils

    nc = build()
    in_maps = shard(inputs)
    res = bass_utils.run_bass_kernel_spmd(
        nc, in_maps, core_ids=list(range(NCORES)), trace=trace)
    bo = np.asarray(inputs["bo"], dtype=np.float32).reshape(OUT_DIM)
    return unshard(res.results, bo), res


def kernel(**inputs):
    return run(inputs)[0]

